# revision 3
# baseline (speedup 1.0000x reference)
"""Trainium2 Bass kernel for the BaseEnergyFormPhysics tet-mesh potential energy.

Strategy (per sharding hint): partition the 2M elements across the 8
NeuronCores.  The host shards conns, gathers the per-element nodal data
(coords/us -> element edge vectors / displacement diffs), and packs it into
dense per-core component planes.  Each core streams its element planes from
HBM and computes, fully on-device, the cross products r_n, det(J), the
displacement gradient G = sum_n w_n (x) r_n, the strain invariants
S = sum(eps^2), T = tr(eps) (scaled by det), and three partial sums

    A = sum S * (1/|det|)^2 * |det|
    B = sum T^2 * (1/|det|)^2 * |det|
    C = sum (sum_n u_nz) * |det|

per partition lane.  The scalar energy is unsharded on the host:
    E = mu/6 * A + lam/12 * B - rho/24 * C  (summed over cores/lanes/chunks)

Degenerate elements (duplicate node -> det exactly 0) produce inf*0 = NaN on
device, matching the NaN the jax reference yields for this input.
"""

import numpy as np
from contextlib import ExitStack

import concourse.bass as bass
import concourse.bacc as bacc
import concourse.tile as tile
import concourse.mybir as mybir
from concourse.bass_utils import run_bass_kernel_spmd

F32 = mybir.dt.float32
AX = mybir.AxisListType
ALU = mybir.AluOpType
ACTF = mybir.ActivationFunctionType

N_CORES = 8
P = 128
N_ELEMS = 2_000_000
E_CORE = N_ELEMS // N_CORES          # 250_000 elements per core
CHUNK_F = 489                        # free-dim columns per chunk
N_CHUNKS = 4
COLS = CHUNK_F * N_CHUNKS            # 1956 columns  (128*1956 = 250368 >= 250000)
E_PAD = P * COLS

# plane order in the packed input:  e1(3) e2(3) e3(3) w1(3) w2(3) w3(3) uqzsum(1)
N_PLANES = 19

_CACHE = {}


def _build(chunk_f=CHUNK_F, n_chunks=N_CHUNKS):
    cols = chunk_f * n_chunks
    nc = bacc.Bacc("TRN2", target_bir_lowering=False, debug=False,
                   num_devices=N_CORES)
    pl = nc.dram_tensor("planes", [P, N_PLANES, cols], F32,
                        kind="ExternalInput").ap()
    out = nc.dram_tensor("out", [P, 3, n_chunks], F32,
                         kind="ExternalOutput").ap()

    with tile.TileContext(nc) as tc, ExitStack() as ctx:
        in_pool = ctx.enter_context(tc.tile_pool(name="inp", bufs=2))
        r_pool = ctx.enter_context(tc.tile_pool(name="rp", bufs=2))
        g_pool = ctx.enter_context(tc.tile_pool(name="gp", bufs=1))
        s_pool = ctx.enter_context(tc.tile_pool(name="sp", bufs=1))
        a_pool = ctx.enter_context(tc.tile_pool(name="ap", bufs=1))

        acc = a_pool.tile([P, 3, n_chunks], F32)

        for c in range(n_chunks):
            F = chunk_f
            inp = in_pool.tile([P, N_PLANES, F], F32)
            nc.sync.dma_start(inp[:], pl[:, :, c * F:(c + 1) * F])

            e_blk = inp[:, 0:9, :]       # (n, comp): e1x..e3z
            w_blk = inp[:, 9:18, :]      # (n, comp): w1x..w3z
            uqz = inp[:, 18, :]          # [P, F]

            # --- cross products r1 = e2 x e3, r2 = e3 x e1, r3 = e1 x e2 (GPSIMD)
            r = r_pool.tile([P, 9, F], F32)     # (n, comp)
            # e vector plane bases within e_blk: e1=0, e2=3, e3=6
            for n, (a, b) in enumerate(((3, 6), (6, 0), (0, 3))):
                for k, (i, j) in enumerate(((1, 2), (2, 0), (0, 1))):
                    # r[n,k] = e_a[i]*e_b[j] - e_a[j]*e_b[i]
                    t1 = r_pool.tile([P, F], F32, tag="crt1")
                    nc.gpsimd.tensor_mul(t1[:], e_blk[:, a + i, :], e_blk[:, b + j, :])
                    t2 = r_pool.tile([P, F], F32, tag="crt2")
                    nc.gpsimd.tensor_mul(t2[:], e_blk[:, a + j, :], e_blk[:, b + i, :])
                    nc.gpsimd.tensor_sub(r[:, 3 * n + k, :], t1[:], t2[:])

            # --- det = e1 . r1   (DVE)
            dp = s_pool.tile([P, 3, F], F32)
            nc.vector.tensor_mul(dp[:], e_blk[:, 0:3, :], r[:, 0:3, :])
            det = s_pool.tile([P, F], F32)
            nc.vector.tensor_reduce(det[:].unsqueeze(2),
                                    dp[:].rearrange("p n f -> p f n"),
                                    axis=AX.X, op=ALU.add)
            adet = s_pool.tile([P, F], F32)
            nc.scalar.activation(adet[:], det[:], ACTF.Abs)
            recip = s_pool.tile([P, F], F32)
            nc.vector.reciprocal_approx_fast(out=recip[:], in_=adet[:])
            recip2 = s_pool.tile([P, F], F32)
            nc.vector.tensor_mul(recip2[:], recip[:], recip[:])

            # --- G[i,d] = sum_n w_n[i] * r_n[d]   (DVE, broadcast APs)
            G = g_pool.tile([P, 3, 3, F], F32)
            gt = g_pool.tile([P, 3, 3, F], F32)
            w1 = w_blk[:, 0:3, :].unsqueeze(2).broadcast_to([P, 3, 3, F])
            r1 = r[:, 0:3, :].unsqueeze(1).broadcast_to([P, 3, 3, F])
            nc.vector.tensor_mul(G[:], w1, r1)
            w2 = w_blk[:, 3:6, :].unsqueeze(2).broadcast_to([P, 3, 3, F])
            r2 = r[:, 3:6, :].unsqueeze(1).broadcast_to([P, 3, 3, F])
            nc.vector.tensor_mul(gt[:], w2, r2)
            nc.vector.tensor_add(G[:], G[:], gt[:])
            w3 = w_blk[:, 6:9, :].unsqueeze(2).broadcast_to([P, 3, 3, F])
            r3 = r[:, 6:9, :].unsqueeze(1).broadcast_to([P, 3, 3, F])
            nc.vector.tensor_mul(gt[:], w3, r3)
            nc.vector.tensor_add(G[:], G[:], gt[:])

            # --- S = sum_i G_ii^2 + 0.5 * sum_{i<j} (G_ij + G_ji)^2
            sqb = s_pool.tile([P, 6, F], F32)
            for i in range(3):
                nc.scalar.activation(sqb[:, i, :], G[:, i, i, :], ACTF.Square)
            for k, (i, j) in enumerate(((0, 1), (0, 2), (1, 2))):
                nc.vector.tensor_add(sqb[:, 3 + k, :], G[:, i, j, :], G[:, j, i, :])
            nc.scalar.activation(sqb[:, 3:6, :], sqb[:, 3:6, :], ACTF.Square,
                                 scale=0.7071067811865476)
            S = s_pool.tile([P, F], F32)
            nc.vector.tensor_reduce(S[:].unsqueeze(2),
                                    sqb[:].rearrange("p n f -> p f n"),
                                    axis=AX.X, op=ALU.add)

            # --- T = tr(G);  T2 = T^2
            tt = s_pool.tile([P, F], F32)
            nc.vector.tensor_add(tt[:], G[:, 0, 0, :], G[:, 1, 1, :])
            T = s_pool.tile([P, F], F32)
            nc.vector.tensor_add(T[:], tt[:], G[:, 2, 2, :])
            T2 = s_pool.tile([P, F], F32)
            nc.scalar.activation(T2[:], T[:], ACTF.Square)

            # --- partial sums (NaN-preserving: X*recip2 is inf/NaN at det==0,
            #     then * adet(=0) -> NaN, matching the reference)
            Srp = s_pool.tile([P, F], F32)
            nc.vector.tensor_mul(Srp[:], S[:], recip2[:])
            Trp = s_pool.tile([P, F], F32)
            nc.vector.tensor_mul(Trp[:], T2[:], recip2[:])

            scr = s_pool.tile([P, F], F32, tag="ttr_scr")
            nc.vector.tensor_mul(scr[:], Srp[:], adet[:])
            nc.vector.tensor_reduce(acc[:, 0, c:c + 1], scr[:],
                                    axis=AX.X, op=ALU.add)
            scr2 = s_pool.tile([P, F], F32, tag="ttr_scr2")
            nc.vector.tensor_mul(scr2[:], Trp[:], adet[:])
            nc.vector.tensor_reduce(acc[:, 1, c:c + 1], scr2[:],
                                    axis=AX.X, op=ALU.add)
            scr3 = s_pool.tile([P, F], F32, tag="ttr_scr3")
            nc.vector.tensor_mul(scr3[:], uqz, adet[:])
            nc.vector.tensor_reduce(acc[:, 2, c:c + 1], scr3[:],
                                    axis=AX.X, op=ALU.add)

        nc.sync.dma_start(out[:], acc[:])

    nc.compile()
    return nc


def _get_nc():
    if "nc" not in _CACHE:
        _CACHE["nc"] = _build()
    return _CACHE["nc"]


def _pack_core(conns_i, coords, us):
    """Gather + pack one core's elements into [P, N_PLANES, COLS] fp32."""
    xs = coords[conns_i]                     # [E,4,3]
    ue = us[conns_i]                         # [E,4,3]
    e = xs[:, 1:4, :] - xs[:, 0:1, :]        # [E,3,3]  (n, comp)
    w = ue[:, 1:4, :] - ue[:, 0:1, :]        # [E,3,3]
    uqzsum = ue[:, :, 2].sum(axis=1)         # [E]

    planes = np.empty((N_PLANES, E_PAD), dtype=np.float32)
    E = conns_i.shape[0]
    planes[0:9, :E] = e.reshape(E, 9).T
    planes[9:18, :E] = w.reshape(E, 9).T
    planes[18, :E] = uqzsum
    # pad with unit tets (det=1, w=0) so padding contributes exactly 0
    pad_e = np.array([1, 0, 0, 0, 1, 0, 0, 0, 1], dtype=np.float32)
    planes[0:9, E:] = pad_e[:, None]
    planes[9:19, E:] = 0.0
    # -> [P, N_PLANES, COLS]
    return np.ascontiguousarray(
        planes.reshape(N_PLANES, P, COLS).transpose(1, 0, 2))


def kernel(params, coords, us, t, conns):
    params = np.asarray(params, np.float32)
    coords = np.asarray(coords, np.float32)
    us = np.asarray(us, np.float32)
    conns = np.asarray(conns)
    lam, mu, rho = (np.float64(params[0]), np.float64(params[1]),
                    np.float64(params[2]))

    nc = _get_nc()
    in_maps = []
    for i in range(N_CORES):
        ci = conns[i * E_CORE:(i + 1) * E_CORE]
        in_maps.append({"planes": _pack_core(ci, coords, us)})

    res = run_bass_kernel_spmd(nc, in_maps, core_ids=list(range(N_CORES)),
                               trace=bool(_CACHE.get("trace", False)))
    _CACHE["last_results"] = res

    total = np.float64(0.0)
    for i in range(N_CORES):
        o = res.results[i]["out"].astype(np.float64)   # [P, 3, N_CHUNKS]
        A = o[:, 0, :].sum()
        B = o[:, 1, :].sum()
        C = o[:, 2, :].sum()
        total += mu / 6.0 * A + lam / 12.0 * B - rho / 24.0 * C
    return np.asarray(total, dtype=np.float32)


# revision 4
# speedup vs baseline: 1.3384x; 1.3384x over previous
"""Trainium2 Bass kernel for the BaseEnergyFormPhysics tet-mesh potential energy.

Strategy (per sharding hint): partition the 2M elements across the 8
NeuronCores.  The host shards conns, gathers the per-element nodal data
(coords/us -> element edge vectors / displacement diffs), and packs it into
dense per-core component planes.  Each core streams its element planes from
HBM and computes, fully on-device, the cross products r_n, det(J), the
displacement gradient G = sum_n w_n (x) r_n, the strain invariants
S = sum(eps^2), T = tr(eps) (scaled by det), and three partial sums

    A = sum S * 1/|det|
    B = sum T^2 * 1/|det|
    C = sum (sum_n u_nz) * |det|

per partition lane.  The scalar energy is unsharded on the host:
    E = mu/6 * A + lam/12 * B - rho/24 * C  (summed over cores/lanes/chunks)

Precision: the det path (edges, cross products r1/r2, det, 1/|det|) runs in
fp32; the strain path (w, G, S, T) runs in fp16, which costs nothing in
accuracy (the error budget is dominated by fp32 near-singular tets) and
doubles DVE throughput.  Degenerate elements (duplicate node -> det exactly
0) produce 1/0 -> NaN on device, matching the NaN the jax reference yields
for this input.
"""

import numpy as np
from contextlib import ExitStack

import concourse.bass as bass
import concourse.bacc as bacc
import concourse.tile as tile
import concourse.mybir as mybir
from concourse.bass_utils import run_bass_kernel_spmd

F32 = mybir.dt.float32
F16 = mybir.dt.float16
AX = mybir.AxisListType
ALU = mybir.AluOpType
ACTF = mybir.ActivationFunctionType

N_CORES = 8
P = 128
N_ELEMS = 2_000_000
E_CORE = N_ELEMS // N_CORES          # 250_000 elements per core
CHUNK_F = 489                        # free-dim columns per chunk
N_CHUNKS = 4
COLS = CHUNK_F * N_CHUNKS            # 1956 columns  (128*1956 = 250368 >= 250000)
E_PAD = P * COLS

# fp32 planes: e1(3) e2(3) e3(3) uqzsum(1);  fp16 planes: w1..w3 (9), e1..e3 (9)
N_P32 = 10
N_P16 = 18

_CACHE = {}


def _build(chunk_f=CHUNK_F, n_chunks=N_CHUNKS):
    cols = chunk_f * n_chunks
    nc = bacc.Bacc("TRN2", target_bir_lowering=False, debug=False,
                   num_devices=N_CORES)
    pl32 = nc.dram_tensor("planes32", [P, N_P32, cols], F32,
                          kind="ExternalInput").ap()
    pl16 = nc.dram_tensor("planes16", [P, N_P16, cols], F16,
                          kind="ExternalInput").ap()
    out = nc.dram_tensor("out", [P, 3, n_chunks], F32,
                         kind="ExternalOutput").ap()

    with tile.TileContext(nc) as tc, ExitStack() as ctx:
        in_pool = ctx.enter_context(tc.tile_pool(name="inp", bufs=2))
        r_pool = ctx.enter_context(tc.tile_pool(name="rp", bufs=2))
        g_pool = ctx.enter_context(tc.tile_pool(name="gp", bufs=1))
        s_pool = ctx.enter_context(tc.tile_pool(name="sp", bufs=2))
        a_pool = ctx.enter_context(tc.tile_pool(name="acp", bufs=1))

        acc = a_pool.tile([P, 3, n_chunks], F32)

        for c in range(n_chunks):
            F = chunk_f
            i32 = in_pool.tile([P, N_P32, F], F32, tag="i32")
            nc.sync.dma_start(i32[:], pl32[:, :, c * F:(c + 1) * F])
            i16 = in_pool.tile([P, N_P16, F], F16, tag="i16")
            nc.sync.dma_start(i16[:], pl16[:, :, c * F:(c + 1) * F])

            e_blk = i32[:, 0:9, :]       # fp32 (n, comp): e1x..e3z
            uqz = i32[:, 9, :]           # [P, F] fp32
            w_blk = i16[:, 0:9, :]       # fp16 (n, comp): w1x..w3z
            eh_blk = i16[:, 9:18, :]     # fp16 copy of e

            # --- cross products: r1 = e2 x e3, r2 = e3 x e1 in fp32 (GPSIMD),
            #     r3 = e1 x e2 in fp16 (DVE, from the fp16 e copy)
            r = r_pool.tile([P, 6, F], F32, tag="r32")      # r1, r2
            rh = r_pool.tile([P, 9, F], F16, tag="rh")      # fp16 r1,r2,r3
            for n, (a, b) in enumerate(((3, 6), (6, 0))):
                for k, (i, j) in enumerate(((1, 2), (2, 0), (0, 1))):
                    t1 = r_pool.tile([P, F], F32, tag="crt1")
                    nc.gpsimd.tensor_mul(t1[:], e_blk[:, a + i, :], e_blk[:, b + j, :])
                    t2 = r_pool.tile([P, F], F32, tag="crt2")
                    nc.gpsimd.tensor_mul(t2[:], e_blk[:, a + j, :], e_blk[:, b + i, :])
                    nc.gpsimd.tensor_sub(r[:, 3 * n + k, :], t1[:], t2[:])
            # fp16 r1, r2 for the G path (ScalarE does the downconvert copy)
            nc.scalar.activation(rh[:, 0:6, :], r[:], ACTF.Copy)
            # r3 (fp16) on DVE:  a=e1(9..11 of i16), b=e2(12..14)
            for k, (i, j) in enumerate(((1, 2), (2, 0), (0, 1))):
                t1h = r_pool.tile([P, F], F16, tag="crt1h")
                nc.vector.tensor_mul(t1h[:], eh_blk[:, 0 + i, :], eh_blk[:, 3 + j, :])
                t2h = r_pool.tile([P, F], F16, tag="crt2h")
                nc.vector.tensor_mul(t2h[:], eh_blk[:, 0 + j, :], eh_blk[:, 3 + i, :])
                nc.vector.tensor_sub(rh[:, 6 + k, :], t1h[:], t2h[:])

            # --- det = e1 . r1   (fp32)
            dp = s_pool.tile([P, 3, F], F32, tag="dp")
            nc.vector.tensor_mul(dp[:], e_blk[:, 0:3, :], r[:, 0:3, :])
            det = s_pool.tile([P, F], F32, tag="det")
            nc.vector.tensor_reduce(det[:].unsqueeze(2),
                                    dp[:].rearrange("p n f -> p f n"),
                                    axis=AX.X, op=ALU.add)
            adet = s_pool.tile([P, F], F32, tag="adet")
            nc.scalar.activation(adet[:], det[:], ACTF.Abs)
            # 1/|det|; exact-zero |det| (duplicate-node elements) -> NaN,
            # which must propagate into A and B to match the reference.
            recip = s_pool.tile([P, F], F32, tag="recip")
            nc.vector.reciprocal_approx_fast(out=recip[:], in_=adet[:])

            # --- G[i,d] = sum_n w_n[i] * r_n[d]   (fp16, broadcast APs)
            G = g_pool.tile([P, 3, 3, F], F16)
            gt = g_pool.tile([P, 3, 3, F], F16)
            w1 = w_blk[:, 0:3, :].unsqueeze(2).broadcast_to([P, 3, 3, F])
            r1b = rh[:, 0:3, :].unsqueeze(1).broadcast_to([P, 3, 3, F])
            nc.vector.tensor_mul(G[:], w1, r1b)
            w2 = w_blk[:, 3:6, :].unsqueeze(2).broadcast_to([P, 3, 3, F])
            r2b = rh[:, 3:6, :].unsqueeze(1).broadcast_to([P, 3, 3, F])
            nc.vector.tensor_mul(gt[:], w2, r2b)
            nc.vector.tensor_add(G[:], G[:], gt[:])
            w3 = w_blk[:, 6:9, :].unsqueeze(2).broadcast_to([P, 3, 3, F])
            r3b = rh[:, 6:9, :].unsqueeze(1).broadcast_to([P, 3, 3, F])
            nc.vector.tensor_mul(gt[:], w3, r3b)
            nc.vector.tensor_add(G[:], G[:], gt[:])

            # --- S = sum_i G_ii^2 + 0.5 * sum_{i<j} (G_ij + G_ji)^2
            sqb = s_pool.tile([P, 6, F], F16, tag="sqb")
            for i in range(3):
                nc.scalar.activation(sqb[:, i, :], G[:, i, i, :], ACTF.Square)
            for k, (i, j) in enumerate(((0, 1), (0, 2), (1, 2))):
                nc.vector.tensor_add(sqb[:, 3 + k, :], G[:, i, j, :], G[:, j, i, :])
            nc.scalar.activation(sqb[:, 3:6, :], sqb[:, 3:6, :], ACTF.Square,
                                 scale=0.7071067811865476)
            S = s_pool.tile([P, F], F32, tag="S")
            nc.vector.tensor_reduce(S[:].unsqueeze(2),
                                    sqb[:].rearrange("p n f -> p f n"),
                                    axis=AX.X, op=ALU.add)

            # --- T = tr(G);  T2 = T^2 (fp32 out)
            tt = s_pool.tile([P, F], F16, tag="tt")
            nc.vector.tensor_add(tt[:], G[:, 0, 0, :], G[:, 1, 1, :])
            T = s_pool.tile([P, F], F16, tag="T")
            nc.vector.tensor_add(T[:], tt[:], G[:, 2, 2, :])
            T2 = s_pool.tile([P, F], F32, tag="T2")
            nc.scalar.activation(T2[:], T[:], ACTF.Square)

            # --- partial sums
            scr = s_pool.tile([P, F], F32, tag="scr")
            nc.vector.tensor_mul(scr[:], S[:], recip[:])
            nc.vector.tensor_reduce(acc[:, 0, c:c + 1], scr[:],
                                    axis=AX.X, op=ALU.add)
            scr2 = s_pool.tile([P, F], F32, tag="scr2")
            nc.vector.tensor_mul(scr2[:], T2[:], recip[:])
            nc.vector.tensor_reduce(acc[:, 1, c:c + 1], scr2[:],
                                    axis=AX.X, op=ALU.add)
            scr3 = s_pool.tile([P, F], F32, tag="scr3")
            nc.vector.tensor_mul(scr3[:], uqz, adet[:])
            nc.vector.tensor_reduce(acc[:, 2, c:c + 1], scr3[:],
                                    axis=AX.X, op=ALU.add)

        nc.sync.dma_start(out[:], acc[:])

    nc.compile()
    return nc


def _get_nc():
    if "nc" not in _CACHE:
        _CACHE["nc"] = _build()
    return _CACHE["nc"]


def _pack_core(conns_i, coords, us):
    """Gather + pack one core's elements into the two plane tensors."""
    xs = coords[conns_i]                     # [E,4,3]
    ue = us[conns_i]                         # [E,4,3]
    e = xs[:, 1:4, :] - xs[:, 0:1, :]        # [E,3,3]  (n, comp)
    w = ue[:, 1:4, :] - ue[:, 0:1, :]        # [E,3,3]
    uqzsum = ue[:, :, 2].sum(axis=1)         # [E]
    E = conns_i.shape[0]
    pad_e = np.array([1, 0, 0, 0, 1, 0, 0, 0, 1], dtype=np.float32)

    p32 = np.empty((N_P32, E_PAD), dtype=np.float32)
    p32[0:9, :E] = e.reshape(E, 9).T
    p32[9, :E] = uqzsum
    p32[0:9, E:] = pad_e[:, None]
    p32[9, E:] = 0.0

    p16 = np.empty((N_P16, E_PAD), dtype=np.float16)
    p16[0:9, :E] = w.reshape(E, 9).T.astype(np.float16)
    p16[9:18, :E] = e.reshape(E, 9).T.astype(np.float16)
    p16[0:9, E:] = 0.0
    p16[9:18, E:] = pad_e.astype(np.float16)[:, None]

    return (np.ascontiguousarray(p32.reshape(N_P32, P, COLS).transpose(1, 0, 2)),
            np.ascontiguousarray(p16.reshape(N_P16, P, COLS).transpose(1, 0, 2)))


def kernel(params, coords, us, t, conns):
    params = np.asarray(params, np.float32)
    coords = np.asarray(coords, np.float32)
    us = np.asarray(us, np.float32)
    conns = np.asarray(conns)
    lam, mu, rho = (np.float64(params[0]), np.float64(params[1]),
                    np.float64(params[2]))

    nc = _get_nc()
    in_maps = []
    for i in range(N_CORES):
        ci = conns[i * E_CORE:(i + 1) * E_CORE]
        a32, a16 = _pack_core(ci, coords, us)
        in_maps.append({"planes32": a32, "planes16": a16})

    res = run_bass_kernel_spmd(nc, in_maps, core_ids=list(range(N_CORES)),
                               trace=bool(_CACHE.get("trace", False)))
    _CACHE["last_results"] = res

    total = np.float64(0.0)
    for i in range(N_CORES):
        o = res.results[i]["out"].astype(np.float64)   # [P, 3, N_CHUNKS]
        A = o[:, 0, :].sum()
        B = o[:, 1, :].sum()
        C = o[:, 2, :].sum()
        total += mu / 6.0 * A + lam / 12.0 * B - rho / 24.0 * C
    return np.asarray(total, dtype=np.float32)


# revision 6
# speedup vs baseline: 1.4527x; 1.0855x over previous
"""Trainium2 Bass kernel for the BaseEnergyFormPhysics tet-mesh potential energy.

Strategy (per sharding hint): partition the 2M elements across the 8
NeuronCores.  The host shards conns, gathers the per-element nodal data
(coords/us -> element edge vectors / displacement diffs), and packs it into
dense per-core component planes.  Each core streams its element planes from
HBM and computes, fully on-device, the cross products r_n, det(J), the
displacement gradient G = sum_n w_n (x) r_n, the strain invariants
S = sum(eps^2), T = tr(eps) (scaled by det), and three partial sums

    A = sum S * 1/|det|
    B = sum T^2 * 1/|det|
    C = sum (sum_n u_nz) * |det|

per partition lane.  The scalar energy is unsharded on the host:
    E = mu/6 * A + lam/12 * B - rho/24 * C  (summed over cores/lanes/chunks)

Precision: the det path (edges, cross products r1/r2, det, 1/|det|) runs in
fp32; the strain path (w, G, S, T) runs in fp16, which costs nothing in
accuracy (the error budget is dominated by fp32 near-singular tets) and
doubles DVE throughput.  Degenerate elements (duplicate node -> det exactly
0) produce 1/0 -> NaN on device, matching the NaN the jax reference yields
for this input.
"""

import numpy as np
from contextlib import ExitStack

import concourse.bass as bass
import concourse.bacc as bacc
import concourse.tile as tile
import concourse.mybir as mybir
from concourse.bass_utils import run_bass_kernel_spmd

F32 = mybir.dt.float32
F16 = mybir.dt.float16
AX = mybir.AxisListType
ALU = mybir.AluOpType
ACTF = mybir.ActivationFunctionType

N_CORES = 8
P = 128
N_ELEMS = 2_000_000
E_CORE = N_ELEMS // N_CORES          # 250_000 elements per core
CHUNK_F = 489                        # free-dim columns per chunk
N_CHUNKS = 4
COLS = CHUNK_F * N_CHUNKS            # 1956 columns  (128*1956 = 250368 >= 250000)
E_PAD = P * COLS

# fp32 planes: e1(3) e2(3) e3(3) uqzsum(1);  fp16 planes: w1..w3 (9), e1..e3 (9)
N_P32 = 10
N_P16 = 18

_CACHE = {}


def _build(chunk_f=CHUNK_F, n_chunks=N_CHUNKS):
    cols = chunk_f * n_chunks
    nc = bacc.Bacc("TRN2", target_bir_lowering=False, debug=False,
                   num_devices=N_CORES)
    pl32 = nc.dram_tensor("planes32", [P, N_P32, cols], F32,
                          kind="ExternalInput").ap()
    pl16 = nc.dram_tensor("planes16", [P, N_P16, cols], F16,
                          kind="ExternalInput").ap()
    out = nc.dram_tensor("out", [P, 3, n_chunks], F32,
                         kind="ExternalOutput").ap()

    with tile.TileContext(nc) as tc, ExitStack() as ctx:
        in_pool = ctx.enter_context(tc.tile_pool(name="inp", bufs=2))
        r_pool = ctx.enter_context(tc.tile_pool(name="rp", bufs=2))
        g_pool = ctx.enter_context(tc.tile_pool(name="gp", bufs=1))
        s_pool = ctx.enter_context(tc.tile_pool(name="sp", bufs=2))
        a_pool = ctx.enter_context(tc.tile_pool(name="acp", bufs=1))

        acc = a_pool.tile([P, 3, n_chunks], F32)

        for c in range(n_chunks):
            F = chunk_f
            i32 = in_pool.tile([P, N_P32, F], F32, tag="i32")
            nc.sync.dma_start(i32[:], pl32[:, :, c * F:(c + 1) * F])
            i16 = in_pool.tile([P, N_P16, F], F16, tag="i16")
            nc.sync.dma_start(i16[:], pl16[:, :, c * F:(c + 1) * F])

            e_blk = i32[:, 0:9, :]       # fp32 (n, comp): e1x..e3z
            uqz = i32[:, 9, :]           # [P, F] fp32
            w_blk = i16[:, 0:9, :]       # fp16 (n, comp): w1x..w3z
            eh_blk = i16[:, 9:18, :]     # fp16 copy of e

            # --- cross products: r1 = e2 x e3 in fp32 (GPSIMD, feeds det);
            #     r2 = e3 x e1, r3 = e1 x e2 in fp16 (DVE, feed only G)
            r = r_pool.tile([P, 3, F], F32, tag="r32")      # r1 only
            rh = r_pool.tile([P, 9, F], F16, tag="rh")      # fp16 r1,r2,r3
            for k, (i, j) in enumerate(((1, 2), (2, 0), (0, 1))):
                t1 = r_pool.tile([P, F], F32, tag="crt1")
                nc.gpsimd.tensor_mul(t1[:], e_blk[:, 3 + i, :], e_blk[:, 6 + j, :])
                t2 = r_pool.tile([P, F], F32, tag="crt2")
                nc.gpsimd.tensor_mul(t2[:], e_blk[:, 3 + j, :], e_blk[:, 6 + i, :])
                nc.gpsimd.tensor_sub(r[:, k, :], t1[:], t2[:])
            # fp16 r1 for the G path (ScalarE does the downconvert copy)
            nc.scalar.activation(rh[:, 0:3, :], r[:], ACTF.Copy)
            # r2, r3 (fp16) on DVE from the fp16 e copy (eh bases: e1=0 e2=3 e3=6)
            for n, (a, b) in enumerate(((6, 0), (0, 3))):
                for k, (i, j) in enumerate(((1, 2), (2, 0), (0, 1))):
                    t1h = r_pool.tile([P, F], F16, tag="crt1h")
                    nc.vector.tensor_mul(t1h[:], eh_blk[:, a + i, :], eh_blk[:, b + j, :])
                    t2h = r_pool.tile([P, F], F16, tag="crt2h")
                    nc.vector.tensor_mul(t2h[:], eh_blk[:, a + j, :], eh_blk[:, b + i, :])
                    nc.vector.tensor_sub(rh[:, 3 + 3 * n + k, :], t1h[:], t2h[:])

            # --- det = e1 . r1   (fp32, GPSIMD)
            dp = s_pool.tile([P, 3, F], F32, tag="dp")
            nc.gpsimd.tensor_mul(dp[:], e_blk[:, 0:3, :], r[:])
            dta = s_pool.tile([P, F], F32, tag="dta")
            nc.gpsimd.tensor_add(dta[:], dp[:, 0, :], dp[:, 1, :])
            det = s_pool.tile([P, F], F32, tag="det")
            nc.gpsimd.tensor_add(det[:], dta[:], dp[:, 2, :])
            adet = s_pool.tile([P, F], F32, tag="adet")
            nc.scalar.activation(adet[:], det[:], ACTF.Abs)
            # 1/|det|; exact-zero |det| (duplicate-node elements) -> NaN,
            # which must propagate into A and B to match the reference.
            recip = s_pool.tile([P, F], F32, tag="recip")
            nc.vector.reciprocal_approx_fast(out=recip[:], in_=adet[:])

            # --- G[i,d] = sum_n w_n[i] * r_n[d]   (fp16, broadcast APs)
            G = g_pool.tile([P, 3, 3, F], F16)
            gt = g_pool.tile([P, 3, 3, F], F16)
            w1 = w_blk[:, 0:3, :].unsqueeze(2).broadcast_to([P, 3, 3, F])
            r1b = rh[:, 0:3, :].unsqueeze(1).broadcast_to([P, 3, 3, F])
            nc.vector.tensor_mul(G[:], w1, r1b)
            w2 = w_blk[:, 3:6, :].unsqueeze(2).broadcast_to([P, 3, 3, F])
            r2b = rh[:, 3:6, :].unsqueeze(1).broadcast_to([P, 3, 3, F])
            nc.vector.tensor_mul(gt[:], w2, r2b)
            nc.vector.tensor_add(G[:], G[:], gt[:])
            w3 = w_blk[:, 6:9, :].unsqueeze(2).broadcast_to([P, 3, 3, F])
            r3b = rh[:, 6:9, :].unsqueeze(1).broadcast_to([P, 3, 3, F])
            nc.vector.tensor_mul(gt[:], w3, r3b)
            nc.vector.tensor_add(G[:], G[:], gt[:])

            # --- S = sum_i G_ii^2 + 0.5 * sum_{i<j} (G_ij + G_ji)^2
            sqb = s_pool.tile([P, 6, F], F16, tag="sqb")
            for i in range(3):
                nc.scalar.activation(sqb[:, i, :], G[:, i, i, :], ACTF.Square)
            for k, (i, j) in enumerate(((0, 1), (0, 2), (1, 2))):
                nc.gpsimd.tensor_add(sqb[:, 3 + k, :], G[:, i, j, :], G[:, j, i, :])
            nc.scalar.activation(sqb[:, 3:6, :], sqb[:, 3:6, :], ACTF.Square,
                                 scale=0.7071067811865476)
            sa = s_pool.tile([P, 3, F], F16, tag="sa")
            nc.vector.tensor_add(sa[:], sqb[:, 0:3, :], sqb[:, 3:6, :])
            s1 = s_pool.tile([P, F], F16, tag="s1")
            nc.vector.tensor_add(s1[:], sa[:, 0, :], sa[:, 1, :])
            S = s_pool.tile([P, F], F32, tag="S")
            nc.vector.tensor_add(S[:], s1[:], sa[:, 2, :])

            # --- T = tr(G);  T2 = T^2 (fp32 out)
            tt = s_pool.tile([P, F], F16, tag="tt")
            nc.gpsimd.tensor_add(tt[:], G[:, 0, 0, :], G[:, 1, 1, :])
            T = s_pool.tile([P, F], F16, tag="T")
            nc.gpsimd.tensor_add(T[:], tt[:], G[:, 2, 2, :])
            T2 = s_pool.tile([P, F], F32, tag="T2")
            nc.scalar.activation(T2[:], T[:], ACTF.Square)

            # --- partial sums (free-dim reduction on ScalarE via accum_out)
            junk = s_pool.tile([P, F], F16, tag="junk")
            scr = s_pool.tile([P, F], F32, tag="scr")
            nc.vector.tensor_mul(scr[:], S[:], recip[:])
            nc.scalar.activation(junk[:], scr[:], ACTF.Copy,
                                 accum_out=acc[:, 0, c:c + 1])
            scr2 = s_pool.tile([P, F], F32, tag="scr2")
            nc.vector.tensor_mul(scr2[:], T2[:], recip[:])
            nc.scalar.activation(junk[:], scr2[:], ACTF.Copy,
                                 accum_out=acc[:, 1, c:c + 1])
            scr3 = s_pool.tile([P, F], F32, tag="scr3")
            nc.gpsimd.tensor_mul(scr3[:], uqz, adet[:])
            nc.scalar.activation(junk[:], scr3[:], ACTF.Copy,
                                 accum_out=acc[:, 2, c:c + 1])

        nc.sync.dma_start(out[:], acc[:])

    nc.compile()
    return nc


def _get_nc():
    if "nc" not in _CACHE:
        _CACHE["nc"] = _build()
    return _CACHE["nc"]


def _pack_core(conns_i, coords, us):
    """Gather + pack one core's elements into the two plane tensors."""
    xs = coords[conns_i]                     # [E,4,3]
    ue = us[conns_i]                         # [E,4,3]
    e = xs[:, 1:4, :] - xs[:, 0:1, :]        # [E,3,3]  (n, comp)
    w = ue[:, 1:4, :] - ue[:, 0:1, :]        # [E,3,3]
    uqzsum = ue[:, :, 2].sum(axis=1)         # [E]
    E = conns_i.shape[0]
    pad_e = np.array([1, 0, 0, 0, 1, 0, 0, 0, 1], dtype=np.float32)

    p32 = np.empty((N_P32, E_PAD), dtype=np.float32)
    p32[0:9, :E] = e.reshape(E, 9).T
    p32[9, :E] = uqzsum
    p32[0:9, E:] = pad_e[:, None]
    p32[9, E:] = 0.0

    p16 = np.empty((N_P16, E_PAD), dtype=np.float16)
    p16[0:9, :E] = w.reshape(E, 9).T.astype(np.float16)
    p16[9:18, :E] = e.reshape(E, 9).T.astype(np.float16)
    p16[0:9, E:] = 0.0
    p16[9:18, E:] = pad_e.astype(np.float16)[:, None]

    return (np.ascontiguousarray(p32.reshape(N_P32, P, COLS).transpose(1, 0, 2)),
            np.ascontiguousarray(p16.reshape(N_P16, P, COLS).transpose(1, 0, 2)))


def kernel(params, coords, us, t, conns):
    params = np.asarray(params, np.float32)
    coords = np.asarray(coords, np.float32)
    us = np.asarray(us, np.float32)
    conns = np.asarray(conns)
    lam, mu, rho = (np.float64(params[0]), np.float64(params[1]),
                    np.float64(params[2]))

    nc = _get_nc()
    in_maps = []
    for i in range(N_CORES):
        ci = conns[i * E_CORE:(i + 1) * E_CORE]
        a32, a16 = _pack_core(ci, coords, us)
        in_maps.append({"planes32": a32, "planes16": a16})

    res = run_bass_kernel_spmd(nc, in_maps, core_ids=list(range(N_CORES)),
                               trace=bool(_CACHE.get("trace", False)))
    _CACHE["last_results"] = res

    total = np.float64(0.0)
    for i in range(N_CORES):
        o = res.results[i]["out"].astype(np.float64)   # [P, 3, N_CHUNKS]
        A = o[:, 0, :].sum()
        B = o[:, 1, :].sum()
        C = o[:, 2, :].sum()
        total += mu / 6.0 * A + lam / 12.0 * B - rho / 24.0 * C
    return np.asarray(total, dtype=np.float32)


# revision 7
# speedup vs baseline: 1.5779x; 1.0862x over previous
"""Trainium2 Bass kernel for the BaseEnergyFormPhysics tet-mesh potential energy.

Strategy (per sharding hint): partition the 2M elements across the 8
NeuronCores.  The host shards conns, gathers the per-element nodal data
(coords/us -> element edge vectors / displacement diffs), and packs it into
dense per-core component planes.  Each core streams its element planes from
HBM and computes, fully on-device, the cross products r_n, det(J), the
displacement gradient G = sum_n w_n (x) r_n, the strain invariants
S = sum(eps^2), T = tr(eps) (scaled by det), and three partial sums

    A = sum S * 1/|det|
    B = sum T^2 * 1/|det|
    C = sum (sum_n u_nz) * |det|

per partition lane.  The scalar energy is unsharded on the host:
    E = mu/6 * A + lam/12 * B - rho/24 * C  (summed over cores/lanes/chunks)

Precision: the det path (edges, cross products r1/r2, det, 1/|det|) runs in
fp32; the strain path (w, G, S, T) runs in fp16, which costs nothing in
accuracy (the error budget is dominated by fp32 near-singular tets) and
doubles DVE throughput.  Degenerate elements (duplicate node -> det exactly
0) produce 1/0 -> NaN on device, matching the NaN the jax reference yields
for this input.
"""

import numpy as np
from contextlib import ExitStack

import concourse.bass as bass
import concourse.bacc as bacc
import concourse.tile as tile
import concourse.mybir as mybir
from concourse.bass_utils import run_bass_kernel_spmd

F32 = mybir.dt.float32
F16 = mybir.dt.float16
AX = mybir.AxisListType
ALU = mybir.AluOpType
ACTF = mybir.ActivationFunctionType

N_CORES = 8
P = 128
N_ELEMS = 2_000_000
E_CORE = N_ELEMS // N_CORES          # 250_000 elements per core
CHUNK_F = 489                        # free-dim columns per chunk
N_CHUNKS = 4
COLS = CHUNK_F * N_CHUNKS            # 1956 columns  (128*1956 = 250368 >= 250000)
E_PAD = P * COLS

# fp32 planes: e1(3) e2(3) e3(3) uqzsum(1);  fp16 planes: w1..w3 (9), e1..e3 (9)
N_P32 = 10
N_P16 = 18

_CACHE = {}


def _build(chunk_f=CHUNK_F, n_chunks=N_CHUNKS):
    cols = chunk_f * n_chunks
    nc = bacc.Bacc("TRN2", target_bir_lowering=False, debug=False,
                   num_devices=N_CORES)
    pl32 = nc.dram_tensor("planes32", [P, N_P32, cols], F32,
                          kind="ExternalInput").ap()
    pl16 = nc.dram_tensor("planes16", [P, N_P16, cols], F16,
                          kind="ExternalInput").ap()
    out = nc.dram_tensor("out", [P, 3, n_chunks], F32,
                         kind="ExternalOutput").ap()

    with tile.TileContext(nc) as tc, ExitStack() as ctx:
        in_pool = ctx.enter_context(tc.tile_pool(name="inp", bufs=2))
        r_pool = ctx.enter_context(tc.tile_pool(name="rp", bufs=2))
        g_pool = ctx.enter_context(tc.tile_pool(name="gp", bufs=1))
        s_pool = ctx.enter_context(tc.tile_pool(name="sp", bufs=2))
        a_pool = ctx.enter_context(tc.tile_pool(name="acp", bufs=1))

        acc = a_pool.tile([P, 3, n_chunks], F32)

        for c in range(n_chunks):
            F = chunk_f
            i32 = in_pool.tile([P, N_P32, F], F32, tag="i32")
            nc.sync.dma_start(i32[:], pl32[:, :, c * F:(c + 1) * F])
            i16 = in_pool.tile([P, N_P16, F], F16, tag="i16")
            nc.sync.dma_start(i16[:], pl16[:, :, c * F:(c + 1) * F])

            e_blk = i32[:, 0:9, :]       # fp32 (n, comp): e1x..e3z
            uqz = i32[:, 9, :]           # [P, F] fp32
            w_blk = i16[:, 0:9, :]       # fp16 (n, comp): w1x..w3z
            eh_blk = i16[:, 9:18, :]     # fp16 copy of e

            # --- cross products: r1 = e2 x e3 in fp32 (GPSIMD, feeds det);
            #     r2 = e3 x e1, r3 = e1 x e2 in fp16 (DVE, feed only G)
            r = r_pool.tile([P, 3, F], F32, tag="r32")      # r1 only
            rh = r_pool.tile([P, 9, F], F16, tag="rh")      # fp16 r1,r2,r3
            for k, (i, j) in enumerate(((1, 2), (2, 0), (0, 1))):
                t1 = r_pool.tile([P, F], F32, tag="crt1")
                nc.gpsimd.tensor_mul(t1[:], e_blk[:, 3 + i, :], e_blk[:, 6 + j, :])
                t2 = r_pool.tile([P, F], F32, tag="crt2")
                nc.gpsimd.tensor_mul(t2[:], e_blk[:, 3 + j, :], e_blk[:, 6 + i, :])
                nc.gpsimd.tensor_sub(r[:, k, :], t1[:], t2[:])
            # fp16 r1 for the G path (ScalarE does the downconvert copy)
            nc.scalar.activation(rh[:, 0:3, :], r[:], ACTF.Copy)
            # r2, r3 (fp16) on DVE from the fp16 e copy (eh bases: e1=0 e2=3 e3=6)
            for n, (a, b) in enumerate(((6, 0), (0, 3))):
                for k, (i, j) in enumerate(((1, 2), (2, 0), (0, 1))):
                    t1h = r_pool.tile([P, F], F16, tag="crt1h")
                    nc.vector.tensor_mul(t1h[:], eh_blk[:, a + i, :], eh_blk[:, b + j, :])
                    t2h = r_pool.tile([P, F], F16, tag="crt2h")
                    nc.vector.tensor_mul(t2h[:], eh_blk[:, a + j, :], eh_blk[:, b + i, :])
                    nc.vector.tensor_sub(rh[:, 3 + 3 * n + k, :], t1h[:], t2h[:])

            # --- det = e1 . r1   (fp32, GPSIMD)
            dp = s_pool.tile([P, 3, F], F32, tag="dp")
            nc.gpsimd.tensor_mul(dp[:], e_blk[:, 0:3, :], r[:])
            dta = s_pool.tile([P, F], F32, tag="dta")
            nc.gpsimd.tensor_add(dta[:], dp[:, 0, :], dp[:, 1, :])
            det = s_pool.tile([P, F], F32, tag="det")
            nc.gpsimd.tensor_add(det[:], dta[:], dp[:, 2, :])
            adet = s_pool.tile([P, F], F32, tag="adet")
            nc.scalar.activation(adet[:], det[:], ACTF.Abs)
            # 1/|det|; exact-zero |det| (duplicate-node elements) -> NaN,
            # which must propagate into A and B to match the reference.
            recip = s_pool.tile([P, F], F32, tag="recip")
            nc.vector.reciprocal_approx_fast(out=recip[:], in_=adet[:])

            # --- G[i,d] = sum_n w_n[i] * r_n[d]   (fp16, broadcast APs)
            G = g_pool.tile([P, 3, 3, F], F16)
            gt = g_pool.tile([P, 3, 3, F], F16)
            w1 = w_blk[:, 0:3, :].unsqueeze(2).broadcast_to([P, 3, 3, F])
            r1b = rh[:, 0:3, :].unsqueeze(1).broadcast_to([P, 3, 3, F])
            nc.vector.tensor_mul(G[:], w1, r1b)
            w2 = w_blk[:, 3:6, :].unsqueeze(2).broadcast_to([P, 3, 3, F])
            r2b = rh[:, 3:6, :].unsqueeze(1).broadcast_to([P, 3, 3, F])
            nc.vector.tensor_mul(gt[:], w2, r2b)
            nc.vector.tensor_add(G[:], G[:], gt[:])
            w3 = w_blk[:, 6:9, :].unsqueeze(2).broadcast_to([P, 3, 3, F])
            r3b = rh[:, 6:9, :].unsqueeze(1).broadcast_to([P, 3, 3, F])
            nc.vector.tensor_mul(gt[:], w3, r3b)
            nc.vector.tensor_add(G[:], G[:], gt[:])

            # --- S = sum_i G_ii^2 + 0.5 * sum_{i<j} (G_ij + G_ji)^2
            sqb = s_pool.tile([P, 6, F], F16, tag="sqb")
            for i in range(3):
                nc.scalar.activation(sqb[:, i, :], G[:, i, i, :], ACTF.Square)
            for k, (i, j) in enumerate(((0, 1), (0, 2), (1, 2))):
                nc.vector.tensor_add(sqb[:, 3 + k, :], G[:, i, j, :], G[:, j, i, :])
            nc.scalar.activation(sqb[:, 3:6, :], sqb[:, 3:6, :], ACTF.Square,
                                 scale=0.7071067811865476)
            sa = s_pool.tile([P, 3, F], F16, tag="sa")
            nc.vector.tensor_add(sa[:], sqb[:, 0:3, :], sqb[:, 3:6, :])
            s1 = s_pool.tile([P, F], F16, tag="s1")
            nc.vector.tensor_add(s1[:], sa[:, 0, :], sa[:, 1, :])
            S = s_pool.tile([P, F], F32, tag="S")
            nc.vector.tensor_add(S[:], s1[:], sa[:, 2, :])

            # --- T = tr(G);  T2 = T^2 (fp32 out)
            tt = s_pool.tile([P, F], F16, tag="tt")
            nc.vector.tensor_add(tt[:], G[:, 0, 0, :], G[:, 1, 1, :])
            T = s_pool.tile([P, F], F16, tag="T")
            nc.vector.tensor_add(T[:], tt[:], G[:, 2, 2, :])
            T2 = s_pool.tile([P, F], F32, tag="T2")
            nc.scalar.activation(T2[:], T[:], ACTF.Square)

            # --- partial sums (free-dim reduction on ScalarE via accum_out)
            junk = s_pool.tile([P, F], F16, tag="junk")
            scr = s_pool.tile([P, F], F32, tag="scr")
            nc.vector.tensor_mul(scr[:], S[:], recip[:])
            nc.scalar.activation(junk[:], scr[:], ACTF.Copy,
                                 accum_out=acc[:, 0, c:c + 1])
            scr2 = s_pool.tile([P, F], F32, tag="scr2")
            nc.vector.tensor_mul(scr2[:], T2[:], recip[:])
            nc.scalar.activation(junk[:], scr2[:], ACTF.Copy,
                                 accum_out=acc[:, 1, c:c + 1])
            scr3 = s_pool.tile([P, F], F32, tag="scr3")
            nc.vector.tensor_mul(scr3[:], uqz, adet[:])
            nc.scalar.activation(junk[:], scr3[:], ACTF.Copy,
                                 accum_out=acc[:, 2, c:c + 1])

        nc.sync.dma_start(out[:], acc[:])

    nc.compile()
    return nc


def _get_nc():
    if "nc" not in _CACHE:
        _CACHE["nc"] = _build()
    return _CACHE["nc"]


def _pack_core(conns_i, coords, us):
    """Gather + pack one core's elements into the two plane tensors."""
    xs = coords[conns_i]                     # [E,4,3]
    ue = us[conns_i]                         # [E,4,3]
    e = xs[:, 1:4, :] - xs[:, 0:1, :]        # [E,3,3]  (n, comp)
    w = ue[:, 1:4, :] - ue[:, 0:1, :]        # [E,3,3]
    uqzsum = ue[:, :, 2].sum(axis=1)         # [E]
    E = conns_i.shape[0]
    pad_e = np.array([1, 0, 0, 0, 1, 0, 0, 0, 1], dtype=np.float32)

    p32 = np.empty((N_P32, E_PAD), dtype=np.float32)
    p32[0:9, :E] = e.reshape(E, 9).T
    p32[9, :E] = uqzsum
    p32[0:9, E:] = pad_e[:, None]
    p32[9, E:] = 0.0

    p16 = np.empty((N_P16, E_PAD), dtype=np.float16)
    p16[0:9, :E] = w.reshape(E, 9).T.astype(np.float16)
    p16[9:18, :E] = e.reshape(E, 9).T.astype(np.float16)
    p16[0:9, E:] = 0.0
    p16[9:18, E:] = pad_e.astype(np.float16)[:, None]

    return (np.ascontiguousarray(p32.reshape(N_P32, P, COLS).transpose(1, 0, 2)),
            np.ascontiguousarray(p16.reshape(N_P16, P, COLS).transpose(1, 0, 2)))


def kernel(params, coords, us, t, conns):
    params = np.asarray(params, np.float32)
    coords = np.asarray(coords, np.float32)
    us = np.asarray(us, np.float32)
    conns = np.asarray(conns)
    lam, mu, rho = (np.float64(params[0]), np.float64(params[1]),
                    np.float64(params[2]))

    nc = _get_nc()
    in_maps = []
    for i in range(N_CORES):
        ci = conns[i * E_CORE:(i + 1) * E_CORE]
        a32, a16 = _pack_core(ci, coords, us)
        in_maps.append({"planes32": a32, "planes16": a16})

    res = run_bass_kernel_spmd(nc, in_maps, core_ids=list(range(N_CORES)),
                               trace=bool(_CACHE.get("trace", False)))
    _CACHE["last_results"] = res

    total = np.float64(0.0)
    for i in range(N_CORES):
        o = res.results[i]["out"].astype(np.float64)   # [P, 3, N_CHUNKS]
        A = o[:, 0, :].sum()
        B = o[:, 1, :].sum()
        C = o[:, 2, :].sum()
        total += mu / 6.0 * A + lam / 12.0 * B - rho / 24.0 * C
    return np.asarray(total, dtype=np.float32)


# revision 8
# speedup vs baseline: 1.5811x; 1.0020x over previous
"""Trainium2 Bass kernel for the BaseEnergyFormPhysics tet-mesh potential energy.

Strategy (per sharding hint): partition the 2M elements across the 8
NeuronCores.  The host shards conns, gathers the per-element nodal data
(coords/us -> element edge vectors / displacement diffs), and packs it into
dense per-core component planes.  Each core streams its element planes from
HBM and computes, fully on-device, the cross products r_n, det(J), the
displacement gradient G = sum_n w_n (x) r_n, the strain invariants
S = sum(eps^2), T = tr(eps) (scaled by det), and three partial sums

    A = sum S * 1/|det|
    B = sum T^2 * 1/|det|
    C = sum (sum_n u_nz) * |det|

per partition lane.  The scalar energy is unsharded on the host:
    E = mu/6 * A + lam/12 * B - rho/24 * C  (summed over cores/lanes/chunks)

Precision: the det path (edges, cross products r1/r2, det, 1/|det|) runs in
fp32; the strain path (w, G, S, T) runs in fp16, which costs nothing in
accuracy (the error budget is dominated by fp32 near-singular tets) and
doubles DVE throughput.  Degenerate elements (duplicate node -> det exactly
0) produce 1/0 -> NaN on device, matching the NaN the jax reference yields
for this input.
"""

import numpy as np
from contextlib import ExitStack

import concourse.bass as bass
import concourse.bacc as bacc
import concourse.tile as tile
import concourse.mybir as mybir
from concourse.bass_utils import run_bass_kernel_spmd

F32 = mybir.dt.float32
F16 = mybir.dt.float16
AX = mybir.AxisListType
ALU = mybir.AluOpType
ACTF = mybir.ActivationFunctionType

N_CORES = 8
P = 128
N_ELEMS = 2_000_000
E_CORE = N_ELEMS // N_CORES          # 250_000 elements per core
CHUNK_F = 489                        # free-dim columns per chunk
N_CHUNKS = 4
COLS = CHUNK_F * N_CHUNKS            # 1956 columns  (128*1956 = 250368 >= 250000)
E_PAD = P * COLS

# fp32 planes: e1(3) e2(3) e3(3) uqzsum(1);  fp16 planes: w1..w3 (9), e1..e3 (9)
N_P32 = 10
N_P16 = 18

_CACHE = {}


def _build(chunk_f=CHUNK_F, n_chunks=N_CHUNKS):
    cols = chunk_f * n_chunks
    nc = bacc.Bacc("TRN2", target_bir_lowering=False, debug=False,
                   num_devices=N_CORES)
    pl32 = nc.dram_tensor("planes32", [P, N_P32, cols], F32,
                          kind="ExternalInput").ap()
    pl16 = nc.dram_tensor("planes16", [P, N_P16, cols], F16,
                          kind="ExternalInput").ap()
    out = nc.dram_tensor("out", [P, 3, n_chunks], F32,
                         kind="ExternalOutput").ap()

    with tile.TileContext(nc) as tc, ExitStack() as ctx:
        in_pool = ctx.enter_context(tc.tile_pool(name="inp", bufs=2))
        r_pool = ctx.enter_context(tc.tile_pool(name="rp", bufs=2))
        g_pool = ctx.enter_context(tc.tile_pool(name="gp", bufs=1))
        s_pool = ctx.enter_context(tc.tile_pool(name="sp", bufs=2))
        a_pool = ctx.enter_context(tc.tile_pool(name="acp", bufs=1))

        acc = a_pool.tile([P, 3, n_chunks], F32)

        for c in range(n_chunks):
            F = chunk_f
            i32 = in_pool.tile([P, N_P32, F], F32, tag="i32")
            nc.sync.dma_start(i32[:], pl32[:, :, c * F:(c + 1) * F])
            i16 = in_pool.tile([P, N_P16, F], F16, tag="i16")
            nc.sync.dma_start(i16[:], pl16[:, :, c * F:(c + 1) * F])

            e_blk = i32[:, 0:9, :]       # fp32 (n, comp): e1x..e3z
            uqz = i32[:, 9, :]           # [P, F] fp32
            w_blk = i16[:, 0:9, :]       # fp16 (n, comp): w1x..w3z
            eh_blk = i16[:, 9:18, :]     # fp16 copy of e

            # --- cross products: r2 = e3 x e1, r3 = e1 x e2 in fp16 (DVE,
            #     feed only G; emitted first -- they depend only on the fp16
            #     DMA); r1 = e2 x e3 in fp32 (GPSIMD, feeds det)
            r = r_pool.tile([P, 3, F], F32, tag="r32")      # r1 only
            rh = r_pool.tile([P, 9, F], F16, tag="rh")      # fp16 r1,r2,r3
            # (eh bases: e1=0 e2=3 e3=6)
            for n, (a, b) in enumerate(((6, 0), (0, 3))):
                for k, (i, j) in enumerate(((1, 2), (2, 0), (0, 1))):
                    t1h = r_pool.tile([P, F], F16, tag="crt1h")
                    nc.vector.tensor_mul(t1h[:], eh_blk[:, a + i, :], eh_blk[:, b + j, :])
                    t2h = r_pool.tile([P, F], F16, tag="crt2h")
                    nc.vector.tensor_mul(t2h[:], eh_blk[:, a + j, :], eh_blk[:, b + i, :])
                    nc.vector.tensor_sub(rh[:, 3 + 3 * n + k, :], t1h[:], t2h[:])
            for k, (i, j) in enumerate(((1, 2), (2, 0), (0, 1))):
                t1 = r_pool.tile([P, F], F32, tag="crt1")
                nc.gpsimd.tensor_mul(t1[:], e_blk[:, 3 + i, :], e_blk[:, 6 + j, :])
                t2 = r_pool.tile([P, F], F32, tag="crt2")
                nc.gpsimd.tensor_mul(t2[:], e_blk[:, 3 + j, :], e_blk[:, 6 + i, :])
                nc.gpsimd.tensor_sub(r[:, k, :], t1[:], t2[:])
            # fp16 r1 for the G path (ScalarE does the downconvert copy)
            nc.scalar.activation(rh[:, 0:3, :], r[:], ACTF.Copy)

            # --- det = e1 . r1   (fp32, GPSIMD)
            dp = s_pool.tile([P, 3, F], F32, tag="dp")
            nc.gpsimd.tensor_mul(dp[:], e_blk[:, 0:3, :], r[:])
            dta = s_pool.tile([P, F], F32, tag="dta")
            nc.gpsimd.tensor_add(dta[:], dp[:, 0, :], dp[:, 1, :])
            det = s_pool.tile([P, F], F32, tag="det")
            nc.gpsimd.tensor_add(det[:], dta[:], dp[:, 2, :])
            adet = s_pool.tile([P, F], F32, tag="adet")
            nc.scalar.activation(adet[:], det[:], ACTF.Abs)
            # 1/|det|; exact-zero |det| (duplicate-node elements) -> NaN,
            # which must propagate into A and B to match the reference.
            recip = s_pool.tile([P, F], F32, tag="recip")
            nc.vector.reciprocal_approx_fast(out=recip[:], in_=adet[:])

            # --- G[i,d] = sum_n w_n[i] * r_n[d]   (fp16, broadcast APs)
            G = g_pool.tile([P, 3, 3, F], F16)
            gt = g_pool.tile([P, 3, 3, F], F16)
            # accumulate the r1 term last: it has the longest dependency
            # chain (GPSIMD crosses -> ScalarE downconvert -> DVE)
            w2 = w_blk[:, 3:6, :].unsqueeze(2).broadcast_to([P, 3, 3, F])
            r2b = rh[:, 3:6, :].unsqueeze(1).broadcast_to([P, 3, 3, F])
            nc.vector.tensor_mul(G[:], w2, r2b)
            w3 = w_blk[:, 6:9, :].unsqueeze(2).broadcast_to([P, 3, 3, F])
            r3b = rh[:, 6:9, :].unsqueeze(1).broadcast_to([P, 3, 3, F])
            nc.vector.tensor_mul(gt[:], w3, r3b)
            nc.vector.tensor_add(G[:], G[:], gt[:])
            w1 = w_blk[:, 0:3, :].unsqueeze(2).broadcast_to([P, 3, 3, F])
            r1b = rh[:, 0:3, :].unsqueeze(1).broadcast_to([P, 3, 3, F])
            nc.vector.tensor_mul(gt[:], w1, r1b)
            nc.vector.tensor_add(G[:], G[:], gt[:])

            # --- S = sum_i G_ii^2 + 0.5 * sum_{i<j} (G_ij + G_ji)^2
            sqb = s_pool.tile([P, 6, F], F16, tag="sqb")
            for i in range(3):
                nc.scalar.activation(sqb[:, i, :], G[:, i, i, :], ACTF.Square)
            for k, (i, j) in enumerate(((0, 1), (0, 2), (1, 2))):
                nc.vector.tensor_add(sqb[:, 3 + k, :], G[:, i, j, :], G[:, j, i, :])
            nc.scalar.activation(sqb[:, 3:6, :], sqb[:, 3:6, :], ACTF.Square,
                                 scale=0.7071067811865476)
            sa = s_pool.tile([P, 3, F], F16, tag="sa")
            nc.vector.tensor_add(sa[:], sqb[:, 0:3, :], sqb[:, 3:6, :])
            s1 = s_pool.tile([P, F], F16, tag="s1")
            nc.vector.tensor_add(s1[:], sa[:, 0, :], sa[:, 1, :])
            S = s_pool.tile([P, F], F32, tag="S")
            nc.vector.tensor_add(S[:], s1[:], sa[:, 2, :])

            # --- T = tr(G);  T2 = T^2 (fp32 out)
            tt = s_pool.tile([P, F], F16, tag="tt")
            nc.vector.tensor_add(tt[:], G[:, 0, 0, :], G[:, 1, 1, :])
            T = s_pool.tile([P, F], F16, tag="T")
            nc.vector.tensor_add(T[:], tt[:], G[:, 2, 2, :])
            T2 = s_pool.tile([P, F], F32, tag="T2")
            nc.scalar.activation(T2[:], T[:], ACTF.Square)

            # --- partial sums (free-dim reduction on ScalarE via accum_out)
            junk = s_pool.tile([P, F], F16, tag="junk")
            scr = s_pool.tile([P, F], F32, tag="scr")
            nc.vector.tensor_mul(scr[:], S[:], recip[:])
            nc.scalar.activation(junk[:], scr[:], ACTF.Copy,
                                 accum_out=acc[:, 0, c:c + 1])
            scr2 = s_pool.tile([P, F], F32, tag="scr2")
            nc.vector.tensor_mul(scr2[:], T2[:], recip[:])
            nc.scalar.activation(junk[:], scr2[:], ACTF.Copy,
                                 accum_out=acc[:, 1, c:c + 1])
            scr3 = s_pool.tile([P, F], F32, tag="scr3")
            nc.vector.tensor_mul(scr3[:], uqz, adet[:])
            nc.scalar.activation(junk[:], scr3[:], ACTF.Copy,
                                 accum_out=acc[:, 2, c:c + 1])

        nc.sync.dma_start(out[:], acc[:])

    nc.compile()
    return nc


def _get_nc():
    if "nc" not in _CACHE:
        _CACHE["nc"] = _build()
    return _CACHE["nc"]


def _pack_core(conns_i, coords, us):
    """Gather + pack one core's elements into the two plane tensors."""
    xs = coords[conns_i]                     # [E,4,3]
    ue = us[conns_i]                         # [E,4,3]
    e = xs[:, 1:4, :] - xs[:, 0:1, :]        # [E,3,3]  (n, comp)
    w = ue[:, 1:4, :] - ue[:, 0:1, :]        # [E,3,3]
    uqzsum = ue[:, :, 2].sum(axis=1)         # [E]
    E = conns_i.shape[0]
    pad_e = np.array([1, 0, 0, 0, 1, 0, 0, 0, 1], dtype=np.float32)

    p32 = np.empty((N_P32, E_PAD), dtype=np.float32)
    p32[0:9, :E] = e.reshape(E, 9).T
    p32[9, :E] = uqzsum
    p32[0:9, E:] = pad_e[:, None]
    p32[9, E:] = 0.0

    p16 = np.empty((N_P16, E_PAD), dtype=np.float16)
    p16[0:9, :E] = w.reshape(E, 9).T.astype(np.float16)
    p16[9:18, :E] = e.reshape(E, 9).T.astype(np.float16)
    p16[0:9, E:] = 0.0
    p16[9:18, E:] = pad_e.astype(np.float16)[:, None]

    return (np.ascontiguousarray(p32.reshape(N_P32, P, COLS).transpose(1, 0, 2)),
            np.ascontiguousarray(p16.reshape(N_P16, P, COLS).transpose(1, 0, 2)))


def kernel(params, coords, us, t, conns):
    params = np.asarray(params, np.float32)
    coords = np.asarray(coords, np.float32)
    us = np.asarray(us, np.float32)
    conns = np.asarray(conns)
    lam, mu, rho = (np.float64(params[0]), np.float64(params[1]),
                    np.float64(params[2]))

    nc = _get_nc()
    in_maps = []
    for i in range(N_CORES):
        ci = conns[i * E_CORE:(i + 1) * E_CORE]
        a32, a16 = _pack_core(ci, coords, us)
        in_maps.append({"planes32": a32, "planes16": a16})

    res = run_bass_kernel_spmd(nc, in_maps, core_ids=list(range(N_CORES)),
                               trace=bool(_CACHE.get("trace", False)))
    _CACHE["last_results"] = res

    total = np.float64(0.0)
    for i in range(N_CORES):
        o = res.results[i]["out"].astype(np.float64)   # [P, 3, N_CHUNKS]
        A = o[:, 0, :].sum()
        B = o[:, 1, :].sum()
        C = o[:, 2, :].sum()
        total += mu / 6.0 * A + lam / 12.0 * B - rho / 24.0 * C
    return np.asarray(total, dtype=np.float32)
